# revision 7
# baseline (speedup 1.0000x reference)
"""Linear attention (elu+1 feature map) Trainium2 kernel, 8-core SPMD.

Reference computation (B=4, L=8192, C=1024, H=8, D=128):
    q,k,v = x@Wq+bq, x@Wk+bk, x@Wv+bv          (per batch)
    Q,K   = elu(q)+1, elu(k)+1                 (per head, D=128)
    KV    = K^T @ (v/L)      [D,D]  per (batch, head)
    Z     = 1/(Q @ K.sum(0) + eps)
    out   = (Q @ KV) * Z * L

Sharding: 8 cores = 4 batches x 2 head-groups.  Core i handles batch i//2
and heads 4*(i%2)..4*(i%2)+4, i.e. columns 512*(i%2)..+512 of Wq/Wk/Wv.
The 1/L and *L cancel exactly, so they are dropped (KV computed from raw v).

Per-core program (all matmuls in float32r: full PE rate, ~1e-4 rel err):
  pass 1 (per 512-row l-block):
    - DMA x block, PE-transpose into xT chunks [c128, l512] (fp32), ACT-copy
      to SBUF with fp32r rounding
    - q^T tiles [cout128, l512] = Wq-chunk^T @ xT  (8 accumulating matmuls);
      feature map elu(x)+1 = max(x+1, min(exp(x),1)); store Q^T to DRAM
    - k, v natural tiles [l128, cout512] = xT-chunk^T @ Wk/Wv; feature map on
      k; v packed as [V_h | ones] columns
    - KV[d, 129] += K_h^T @ [V_h | 1]  accumulated in pinned PSUM (col 128
      becomes K.sum)
  pass 2 (per l-block): out[l,129] = Q_h^T-chunk^T @ [KV_h | Ksum_h]; DVE
    scales cols 0..128 by 1/(col128 + eps); DMA out.
"""

import numpy as np

import concourse.bass as bass
import concourse.mybir as mybir
import concourse.tile as tile
from concourse.bass_utils import run_bass_kernel_spmd
from concourse.masks import make_identity

F32 = mybir.dt.float32
F32R = mybir.dt.float32r
AF = mybir.ActivationFunctionType
ALU = mybir.AluOpType

B, L, C = 4, 8192, 1024
H, D = 8, 128
N_CORES = 8
HPC = 4              # heads per core
COUT = HPC * D       # 512 channels per core
LB = 512             # l-block size
NLB = L // LB        # 16
NCC = C // 128       # 8 contraction chunks
EPS = 1e-6


def _split_multi_waits(nc):
    """This container's walrus encodes at most ONE sync wait per ISA
    instruction ("Too many sync wait commands" otherwise).  Hoist all but the
    last wait of each instruction onto standalone InstEventSemaphore
    instructions (the same type Tile's barriers use) right before it in the
    same engine stream — semantics are identical."""
    for f in nc.m.functions:
        for bb in f.blocks:
            insts = bb.instructions
            if not any(
                i.sync_info is not None and len(i.sync_info.on_wait) > 1
                for i in insts
            ):
                continue
            new_insts = []
            for inst in insts:
                si = inst.sync_info
                if si is not None and len(si.on_wait) > 1:
                    waits = list(si.on_wait)
                    for k, w in enumerate(waits[:-1]):
                        new_insts.append(
                            mybir.InstEventSemaphore(
                                name=f"{inst.name}_hw{k}",
                                engine=inst.engine,
                                ins=[],
                                outs=[],
                                sync_info=mybir.SyncInfo(on_wait=[w], on_update=[]),
                            )
                        )
                    inst.sync_info = mybir.SyncInfo(
                        on_wait=[waits[-1]], on_update=list(si.on_update)
                    )
                new_insts.append(inst)
            bb.instructions[:] = new_insts


def _elu1(nc, pools, psum_in, out_f32r, bias_exp=0.0, bias_add1=1.0):
    """out = elu(psum_in + bias) + 1 = max(psum_in + bias + 1, min(exp(psum_in
    + bias), 1)), written to an fp32r SBUF tile (rounding producer)."""
    epool, mpool = pools
    shp = [psum_in.shape[0], psum_in.shape[-1]]
    e = epool.tile(shp, F32, tag="elu_e")
    nc.scalar.activation(e[:], psum_in, AF.Exp, bias=bias_exp)
    m = mpool.tile(shp, F32, tag="elu_m")
    nc.vector.tensor_scalar_min(m[:], e[:], 1.0)
    nc.vector.scalar_tensor_tensor(
        out_f32r, psum_in, bias_add1, m[:], ALU.add, ALU.max
    )


def build_program(with_bias, for_hw=True):
    nc = bass.Bass(
        "TRN2", target_bir_lowering=False, debug=False, num_devices=N_CORES
    )
    x_d = nc.dram_tensor("x", [L, C], F32, kind="ExternalInput").ap()
    wq_d = nc.dram_tensor("wq", [C, COUT], F32, kind="ExternalInput").ap()
    wk_d = nc.dram_tensor("wk", [C, COUT], F32, kind="ExternalInput").ap()
    wv_d = nc.dram_tensor("wv", [C, COUT], F32, kind="ExternalInput").ap()
    if with_bias:
        # bq / bq+1 consumed as per-partition scalars; bk/bv as K=1 matmul rows
        bq_d = nc.dram_tensor("bq", [COUT], F32, kind="ExternalInput").ap()
        bq1_d = nc.dram_tensor("bq1", [COUT], F32, kind="ExternalInput").ap()
        bk_d = nc.dram_tensor("bk", [COUT], F32, kind="ExternalInput").ap()
        bv_d = nc.dram_tensor("bv", [COUT], F32, kind="ExternalInput").ap()
    qT_d = nc.dram_tensor("qT_scratch", [COUT, L], F32R).ap()
    out_d = nc.dram_tensor("out", [L, COUT], F32, kind="ExternalOutput").ap()

    with tile.TileContext(nc) as tc:
        with (
            tc.tile_pool(name="consts", bufs=1) as consts,
            tc.tile_pool(name="weights", bufs=1) as weights,
            tc.tile_pool(name="kv_sb", bufs=1) as kv_sb,
            tc.tile_pool(name="ps_kv", bufs=1, space="PSUM") as ps_kv,
        ):
            identity = consts.tile([128, 128], F32)
            make_identity(nc, identity)
            ones_col = consts.tile([128, HPC], F32)
            nc.gpsimd.memset(ones_col[:], 1.0)
            zeros_col = consts.tile([128, HPC], F32)
            nc.gpsimd.memset(zeros_col[:], 0.0)

            # weights -> SBUF fp32r (DVE copy is the fp32r rounding producer)
            wts = []
            wstage = consts.tile([128, NCC, COUT], F32)
            for name, w_d in (("wq", wq_d), ("wk", wk_d), ("wv", wv_d)):
                wt = weights.tile([128, NCC, COUT], F32R, tag=f"w_{name}")
                nc.gpsimd.dma_start(
                    wstage[:], w_d.rearrange("(cc cp) n -> cp cc n", cp=128)
                )
                nc.vector.tensor_copy(wt[:], wstage[:])
                wts.append(wt)
            wq_t, wk_t, wv_t = wts

            if with_bias:
                bq_t = consts.tile([128, HPC], F32)
                nc.gpsimd.dma_start(bq_t[:], bq_d.rearrange("(co p) -> p co", p=128))
                bq1_t = consts.tile([128, HPC], F32)
                nc.gpsimd.dma_start(bq1_t[:], bq1_d.rearrange("(co p) -> p co", p=128))
                bk_row = consts.tile([1, COUT], F32)
                nc.gpsimd.dma_start(bk_row[:], bk_d[None, :])
                bk_row_r = consts.tile([1, COUT], F32R)
                nc.vector.tensor_copy(bk_row_r[:], bk_row[:])
                bv_row = consts.tile([1, COUT], F32)
                nc.gpsimd.dma_start(bv_row[:], bv_d[None, :])
                bv_row_r = consts.tile([1, COUT], F32R)
                nc.vector.tensor_copy(bv_row_r[:], bv_row[:])
                ones_row = consts.tile([1, 128], F32)
                nc.gpsimd.memset(ones_row[:], 1.0)
                ones_row_r = consts.tile([1, 128], F32R)
                nc.vector.tensor_copy(ones_row_r[:], ones_row[:])

            # pinned KV accumulators: 2 banks, 2 heads each: [d128, 2*129]
            kv_ps = [ps_kv.tile([128, 260], F32, name=f"kv{i}", tag=f"kv{i}") for i in range(2)]

            # ---------------- pass 1 ----------------
            with (
                tc.tile_pool(name="xin", bufs=2) as xin,
                tc.tile_pool(name="xtp", bufs=2) as xtp,
                tc.tile_pool(name="elu_e", bufs=3) as epool,
                tc.tile_pool(name="elu_m", bufs=3) as mpool,
                tc.tile_pool(name="ktile", bufs=2) as kpool,
                tc.tile_pool(name="vtile", bufs=2) as vpool,
                tc.tile_pool(name="qout", bufs=3) as qpool,
                tc.tile_pool(name="ps_t", bufs=2, space="PSUM") as ps_t,
                tc.tile_pool(name="ps_q", bufs=2, space="PSUM") as ps_q,
                tc.tile_pool(name="ps_k", bufs=1, space="PSUM") as ps_k,
                tc.tile_pool(name="ps_v", bufs=1, space="PSUM") as ps_v,
            ):
                for lb in range(NLB):
                    xt = xin.tile([128, 4, C], F32, tag="xt")
                    nc.gpsimd.dma_start(
                        xt[:],
                        x_d[lb * LB:(lb + 1) * LB, :].rearrange(
                            "(ls lp) c -> lp ls c", lp=128
                        ),
                    )
                    xT = xtp.tile([128, NCC, LB], F32R, tag="xT")
                    for cc in range(NCC):
                        pt = ps_t.tile([128, LB], F32, tag="pt")
                        for ls in range(4):
                            nc.tensor.transpose(
                                pt[:, ls * 128:(ls + 1) * 128],
                                xt[:, ls, cc * 128:(cc + 1) * 128],
                                identity[:],
                            )
                        nc.scalar.copy(xT[:, cc, :], pt[:])

                    # q^T per head (cout chunk of 128)
                    for co in range(HPC):
                        pq = ps_q.tile([128, LB], F32, tag="pq")
                        for cc in range(NCC):
                            nc.tensor.matmul(
                                pq[:],
                                wq_t[:, cc, co * 128:(co + 1) * 128],
                                xT[:, cc, :],
                                start=(cc == 0),
                                stop=(cc == NCC - 1),
                            )
                        qTt = qpool.tile([128, LB], F32R, tag="qTt")
                        if with_bias:
                            _elu1(nc, (epool, mpool), pq[:], qTt[:],
                                  bias_exp=bq_t[:, co:co + 1],
                                  bias_add1=bq1_t[:, co:co + 1])
                        else:
                            _elu1(nc, (epool, mpool), pq[:], qTt[:])
                        nc.gpsimd.dma_start(
                            qT_d[co * 128:(co + 1) * 128, lb * LB:(lb + 1) * LB],
                            qTt[:],
                        )

                    # k, v natural + KV accumulation, per l-subtile
                    for ls in range(4):
                        pk = ps_k.tile([128, COUT], F32, tag="pk")
                        for cc in range(NCC):
                            nc.tensor.matmul(
                                pk[:],
                                xT[:, cc, ls * 128:(ls + 1) * 128],
                                wk_t[:, cc, :],
                                start=(cc == 0),
                                stop=(cc == NCC - 1) if not with_bias else False,
                            )
                        if with_bias:
                            nc.tensor.matmul(
                                pk[:], ones_row_r[:], bk_row_r[:],
                                start=False, stop=True,
                            )
                        kt = kpool.tile([128, COUT], F32R, tag="kt")
                        _elu1(nc, (epool, mpool), pk[:], kt[:])

                        pv = ps_v.tile([128, COUT], F32, tag="pv")
                        for cc in range(NCC):
                            nc.tensor.matmul(
                                pv[:],
                                xT[:, cc, ls * 128:(ls + 1) * 128],
                                wv_t[:, cc, :],
                                start=(cc == 0),
                                stop=(cc == NCC - 1) if not with_bias else False,
                            )
                        if with_bias:
                            nc.tensor.matmul(
                                pv[:], ones_row_r[:], bv_row_r[:],
                                start=False, stop=True,
                            )
                        vt = vpool.tile([128, HPC, 130], F32R, tag="vt")
                        for h in range(HPC):
                            nc.vector.tensor_copy(
                                vt[:, h, 0:128], pv[:, h * 128:(h + 1) * 128]
                            )
                        nc.vector.tensor_copy(vt[:, :, 128], ones_col[:])
                        nc.vector.tensor_copy(vt[:, :, 129], zeros_col[:])

                        for h in range(HPC):
                            # one accumulation group per PSUM bank (2 heads
                            # share a bank): start zeroes the whole bank
                            nc.tensor.matmul(
                                kv_ps[h // 2][:, (h % 2) * 130:(h % 2) * 130 + 130],
                                kt[:, h * 128:(h + 1) * 128],
                                vt[:, h, :],
                                start=(lb == 0 and ls == 0 and h % 2 == 0),
                                stop=(lb == NLB - 1 and ls == 3 and h % 2 == 1),
                            )

            # KV psum -> SBUF fp32r
            kvs = kv_sb.tile([128, HPC, 130], F32R)
            for h in range(HPC):
                nc.vector.tensor_copy(
                    kvs[:, h, :], kv_ps[h // 2][:, (h % 2) * 130:(h % 2) * 130 + 130]
                )

            # ---------------- pass 2 ----------------
            with (
                tc.tile_pool(name="qin", bufs=2) as qin,
                tc.tile_pool(name="ostage", bufs=3) as opool,
                tc.tile_pool(name="ztile", bufs=4) as zpool,
                tc.tile_pool(name="ps_o", bufs=4, space="PSUM") as ps_o,
            ):
                for lb in range(NLB):
                    qt = qin.tile([128, HPC, LB], F32R, tag="qt")
                    nc.gpsimd.dma_start(
                        qt[:],
                        qT_d[:, lb * LB:(lb + 1) * LB].rearrange(
                            "(h p) l -> p h l", p=128
                        ),
                    )
                    for ls in range(4):
                        ost = opool.tile([128, COUT], F32, tag="ost")
                        for h in range(HPC):
                            po = ps_o.tile([128, 130], F32, tag="po")
                            nc.tensor.matmul(
                                po[:],
                                qt[:, h, ls * 128:(ls + 1) * 128],
                                kvs[:, h, :],
                                start=True,
                                stop=True,
                            )
                            z = zpool.tile([128, 1], F32, tag="z")
                            nc.vector.tensor_scalar_add(z[:], po[:, 128:129], EPS)
                            nc.vector.reciprocal(z[:], z[:])
                            nc.vector.tensor_scalar_mul(
                                ost[:, h * 128:(h + 1) * 128], po[:, 0:128], z[:]
                            )
                        nc.gpsimd.dma_start(
                            out_d[lb * LB + ls * 128: lb * LB + (ls + 1) * 128, :],
                            ost[:],
                        )

    if for_hw:
        _split_multi_waits(nc)
        nc.finalize()
    return nc


_CACHE = {}


def _get_program(with_bias):
    if with_bias not in _CACHE:
        _CACHE[with_bias] = build_program(with_bias)
    return _CACHE[with_bias]


def kernel(x, Wq, bq, Wk, bk, Wv, bv):
    x = np.ascontiguousarray(np.asarray(x, dtype=np.float32))
    Wq = np.asarray(Wq, dtype=np.float32)
    Wk = np.asarray(Wk, dtype=np.float32)
    Wv = np.asarray(Wv, dtype=np.float32)
    bq = np.asarray(bq, dtype=np.float32)
    bk = np.asarray(bk, dtype=np.float32)
    bv = np.asarray(bv, dtype=np.float32)

    with_bias = bool(np.any(bq) or np.any(bk) or np.any(bv))
    nc = _get_program(with_bias)

    in_maps = []
    for i in range(N_CORES):
        n, g = divmod(i, 2)
        sl = slice(g * COUT, (g + 1) * COUT)
        m = {
            "x": x[n],
            "wq": np.ascontiguousarray(Wq[:, sl]),
            "wk": np.ascontiguousarray(Wk[:, sl]),
            "wv": np.ascontiguousarray(Wv[:, sl]),
        }
        if with_bias:
            m["bq"] = np.ascontiguousarray(bq[sl])
            m["bq1"] = np.ascontiguousarray(bq[sl] + 1.0)
            m["bk"] = np.ascontiguousarray(bk[sl])
            m["bv"] = np.ascontiguousarray(bv[sl])
        in_maps.append(m)

    res = run_bass_kernel_spmd(nc, in_maps, core_ids=list(range(N_CORES)))

    out = np.empty((B, L, C), dtype=np.float32)
    for i in range(N_CORES):
        n, g = divmod(i, 2)
        out[n, :, g * COUT:(g + 1) * COUT] = res.results[i]["out"]
    return out
